# revision 21
# baseline (speedup 1.0000x reference)
"""Trainium2 Bass kernel for nn_Averager (pooling, 3-level box-average).

Math (verified vs reference): per sample, with input x[n, i, c] where
n = (n5 n4 n3 n2 n1 n0) base-4 digits, c = (c2 c1 c0) base-4 digits:
  out[:, :, 0, :] = x[:, :, 0, :]
  out1[n, c] = E[n4, n2, c2, c0, n0, c1],
      E[r5, r4, r3, r0; g2, g1] = mean over (n2, n1, c0) of x1
  out2[n, c] = G[c2, c1, c0],
      G[p, q, r] = mean over (n4, n3, n1, n0, c1, c0) of x2 with
      (n5, c2in, n2) = (p, q, r)

Sharding: data-parallel over batch, 4 samples per core on 8 cores,
processed as 2 groups of 2 samples.

Layout (pair-contiguous): SBUF partition p = b*64 + n//64 =
(b, n5, n4, n3); free j = n % 64 = 16*n2 + 4*n1 + n0, row (i, c).
A 6MB group is contiguous in DRAM and per-partition contiguous in
SBUF: each group is ONE 2-D in-DMA split in 2 j-halves.

DMA facts (measured): the 16 per-core queue engines cap at ~27GB/s
each regardless of descriptor size >=12KB (460/908/1814ns for
12/24/48KB), and the cost is set by the DRAM-side bytes (casting
DMAs don't reduce it).  Total in+out traffic 25.2MB/core = ~59us of
queue time — the pipeline floor.  256B-run patterns (per-level
regions) are descriptor-bound and hopeless.

ALL DMAs (in AND out) ride the single gpsimd SWDGE ring (Q0): the
ring is strict FIFO per queue engine, so output descriptors enqueue
behind the input stream and drain seamlessly the moment inputs
finish — no gate DMA, no HWDGE inter-chunk wait-for-complete stalls
(the old Sync HWDGE path serialized chunk N+1's enqueue on chunk N's
full completion, leaving 3-6us bubbles), and a straggling input
queue self-adjusts (out descriptors just queue up behind it).

More measured queue-engine facts (probe.py / microbench.py):
 - A DMA with descriptor count c spreads over min(16, 2^v2(c))
   queues (62 descs -> 2 queues at 31 each = serialized disaster!).
   Keep desc counts multiples of 16; uniform 1/16-per-queue shares
   are therefore unavoidable.  Each DMA also appends a 4B
   completion-sem descriptor on EVERY queue it used.
 - Setting qPoolDynamic num_queues=15 collapses per-queue bandwidth
   to ~17GB/s — do not touch.
 - E79 also hosts the dynamic rings' doorbell work and goes
   intermittently ~25% slow per BYTE (not per descriptor), +10-15us
   stragglers on a time-varying fraction of runs (external
   interference; always E79; hits all designs equally, unfixable in
   software since shares are uniform).
 - Startup is ~9.2us: ~7.4us Bass-init preamble (sem range clears,
   engine table loads, const memsets + all-engine barrier) before
   any user instruction, then ~0.7us/DMA descriptor-gen on gpsimd
   and ~1.4us doorbell->first-data latency.
 - The Tile epilogue sem-clear chain costs ~52ns per allocated
   semaphore serialized per engine — keep instruction count low
   (this is why selector expansion is 3 fused broadcast ops, not 20,
   and out-chunks are (0),(1),(2,3) not 4 quarters).
 - Healthy steady state after BIR surgery (see _build_nc tail):
   ~69.7us = 6.8us compiler preamble (gauge exec_time counts from the
   first branch at t~30; the ~3us per-engine sem-range clears are
   NEFF-level, untouchable from BIR) + 0.65 desc-gen + 0.45 doorbell
   + 60.1us gapless data window + ~1.3us ring-empty drain detection.
   gauge's exec endpoint lands just after the last real work, so the
   exit path is stripped to Pool's ring drain ALONE (no exit barriers
   at all): the drain's hardware ring-empty check (~0.93us detection
   latency after the last trailer) beats the barrier's semaphore-
   propagation path, and NEFF completion still waits for every
   engine's stream end.  Healthy exec: ~69.4-69.7us.

Engine split per group:
  DVE : L1 lane-local sums (u0/u1/w/h1/h2 -> A, bf16) and L2 sums
        (r01/r23 -> A2, bf16; split by j-half so the r01 reduce fills
        the stall between the two in-DMA halves), plus half of g1's
        PSUM evacuations (DVE is idle after stage-A; ACT alone made
        the last-chunk evac the exec tail).
  PE  : 16 L1 routing matmuls + 4 L2 reduce matmuls, 1-pass (lhsT =
        fp8e5 selectors — 1/64 and 1/4096 are exact powers of two —
        with bf16 rhs; fp32 would take 2 LDWEIGHTS+MATMUL passes).
        The L1 matmuls write PSUM through a strided out-AP so PSUM
        free = (n2, n0, c) matches the output interleave.
  ACT : PSUM evacuations (256B-contiguous runs, n1 via step-0 src dim)
        and the on-chip broadcast of the single L2 row G into all 64
        j-rows — runs parallel to DVE working on the next group.
Phase-1 (stage-A + matmuls, both groups) is emitted before phase-2
(evac + flush) so the framework's stage-reset barriers — which wait
on all earlier DMAs — never land inside the critical DVE chain.
Outputs are assembled IN-PLACE into the input tile (L0 rows pass
through untouched) and flushed in chunks (n2=0), (1), (2,3) as each
chunk's evac+broadcast completes: the first quarter's single 2us
evac is ready right as the input stream drains (12KB descriptors
are bandwidth-neutral), eliminating the transition bubble.
"""

import numpy as np

N_CORES = 8
B_FULL = 32
B_CORE = B_FULL // N_CORES  # 4
N = 4096
LVL = 3
C = 64


def _make_s12():
    """Compact selector factors, expanded on-chip (327KB -> 35KB DMA).

    S1[:, blk] = D1 * masks1[:, blk] and S2[:, c2o] = E2 * masks2[:, c2o]
    where the masks depend only on the partition index k.  All values are
    exact powers of two (or 0/1) so the fp8e5 multiplies are exact.

    Layout [128, 276] fp8e5: [0:128] D1 = (b==bm & k5==m4)/64,
    [128:256] E2 = (b==bm)/4096, [256:272] masks1 (blk = n2o*4+c2o),
    [272:276] masks2 (c2o).
    """
    import ml_dtypes

    k = np.arange(128)
    b, k5, k4, k3 = k >> 6, (k >> 4) & 3, (k >> 2) & 3, k & 3
    m = np.arange(128)
    bm, m4 = m >> 6, (m >> 2) & 3
    D1 = ((b[:, None] == bm[None, :]) & (k5[:, None] == m4[None, :])).astype(
        np.float32
    ) / 64.0
    E2 = (b[:, None] == bm[None, :]).astype(np.float32) / 4096.0
    masks1 = np.zeros((128, 16), np.float32)
    for n2o in range(4):
        for c2o in range(4):
            masks1[:, n2o * 4 + c2o] = ((k4 == n2o) & (k3 == c2o)).astype(
                np.float32
            )
    masks2 = np.zeros((128, 4), np.float32)
    for c2o in range(4):
        masks2[:, c2o] = (k5 == c2o).astype(np.float32)
    return np.ascontiguousarray(
        np.concatenate([D1, E2, masks1, masks2], axis=1).astype(
            ml_dtypes.float8_e5m2
        )
    )


def _build_nc():
    import concourse.bass as bass
    import concourse.tile as tile
    from concourse import mybir

    dt = mybir.dt.float32
    bf = mybir.dt.bfloat16
    f8 = mybir.dt.float8e5
    X = mybir.AxisListType.X
    ADD = mybir.AluOpType.add

    from concourse import bacc
    nc = bacc.Bacc()
    x = nc.declare_dram_parameter("x", [B_CORE, N, LVL, C], dt, isOutput=False)
    s12 = nc.declare_dram_parameter("s12", [128, 276], f8, isOutput=False)
    out = nc.declare_dram_parameter("out", [B_CORE, N, LVL, C], dt, isOutput=True)

    NG = B_CORE // 2

    with tile.TileContext(nc) as tc:
        with (
            tc.tile_pool(name="consts", bufs=1) as cpool,
            tc.tile_pool(name="xin", bufs=2) as xpool,
            tc.tile_pool(name="tmp", bufs=1) as tpool,
            tc.tile_pool(name="psum", bufs=2, space="PSUM") as ppool,
        ):
            # ---- all input DMAs upfront: x group 0 halves, selectors,
            # x group 1 halves, drained back-to-back in this order
            # (selector-first was tried and measured ~1.5us WORSE on
            # healthy runs — keep s12c third) ----
            # One full-tile DMA per group: 48KB descriptors run at
            # 27.10GB/s vs 26.98 at 24KB (2x911=1822 vs 1814ns) — the
            # rate rises slightly with descriptor size, worth ~0.13us
            # across the in-stream.  Stage-A then waits for the whole
            # tile (not a j-half), which still leaves ~1us of margin
            # before the first out-chunk's ring slot.
            xts_ = []
            for g in range(NG):
                xt = xpool.tile([128, 12288], dt, tag="xt")
                xsrc = x[2 * g:2 * g + 2].rearrange(
                    "b (ph j) i c -> (b ph) (j i c)", ph=64
                )
                nc.gpsimd.dma_start(xt[:], xsrc[:])
                xts_.append(xt)
                if g == 0:
                    s12c = cpool.tile([128, 276], f8, tag="s12c")
                    nc.gpsimd.dma_start(s12c[:], s12[:])

            # ---- expand compact selector factors on DVE (idle until the
            # first x half lands): S1 blk = D1 * mask1[blk], S2 c2o =
            # E2 * mask2[c2o].  Exact powers of two -> fp8e5 exact. ----
            ssb = cpool.tile([128, 2560], f8, tag="ssb")
            s1sb = ssb[:, 0:2048]
            s2sb = ssb[:, 2048:2560]
            maskf = cpool.tile([128, 20], dt, tag="maskf")
            nc.vector.tensor_copy(maskf[:], s12c[:, 256:276])
            nc.vector.tensor_mul(
                s1sb.rearrange("p (blk m) -> p blk m", blk=16, m=128),
                s12c[:, 0:128].rearrange(
                    "p (o m) -> p o m", o=1).broadcast_to((128, 16, 128)),
                maskf[:, 0:16].rearrange(
                    "p (blk o) -> p blk o", o=1).broadcast_to((128, 16, 128)),
            )
            nc.vector.tensor_mul(
                s2sb.rearrange("p (blk m) -> p blk m", blk=4, m=128),
                s12c[:, 128:256].rearrange(
                    "p (o m) -> p o m", o=1).broadcast_to((128, 4, 128)),
                maskf[:, 16:20].rearrange(
                    "p (blk o) -> p blk o", o=1).broadcast_to((128, 4, 128)),
            )

            # ---- phase 1 (both groups): DVE stage-A + PE matmuls.
            # Emitted before any out-DMA so the framework's stage-reset
            # barriers (which wait on all earlier DMAs) never insert an
            # out-DMA wait into the critical DVE chain. ----
            c1ps, gps = [], []
            for g in range(NG):
                xt = xts_[g]
                v = xt[:].rearrange(
                    "p (n2 n1 n0 i c) -> p n2 n1 n0 i c",
                    n2=4, n1=4, n0=4, i=3, c=64,
                )
                xw = xt[:].rearrange(
                    "p (j i c2 cc) -> p j i c2 cc", j=64, i=3, c2=4, cc=16
                )

                # ---- stage A per j-half (each needs only that in-half):
                # L1: u = fold n2-pairs of i=1 rows
                # L2: r = fold (c1 c0) of i=2 rows ----
                u0 = tpool.tile([128, 1024], dt, tag="u0")
                nc.vector.tensor_add(
                    u0[:].rearrange("p (n1 n0 c) -> p n1 n0 c", n1=4, n0=4, c=64),
                    v[:, 0, :, :, 1, :], v[:, 1, :, :, 1, :],
                )
                r01 = tpool.tile([128, 128], dt, tag="r01")
                nc.vector.tensor_reduce(
                    r01[:].rearrange("p (j c2) -> p j c2", j=32, c2=4),
                    xw[:, 0:32, 2, :, :],
                    axis=X, op=ADD,
                )
                u1 = tpool.tile([128, 1024], dt, tag="u1")
                nc.vector.tensor_add(
                    u1[:].rearrange("p (n1 n0 c) -> p n1 n0 c", n1=4, n0=4, c=64),
                    v[:, 2, :, :, 1, :], v[:, 3, :, :, 1, :],
                )
                r23 = tpool.tile([128, 128], dt, tag="r23")
                nc.vector.tensor_reduce(
                    r23[:].rearrange("p (j c2) -> p j c2", j=32, c2=4),
                    xw[:, 32:64, 2, :, :],
                    axis=X, op=ADD,
                )

                # ---- cross-half folds -> A (L1) and A2 (L2), bf16 ----
                w = tpool.tile([128, 1024], dt, tag="w")
                nc.vector.tensor_add(w[:], u0[:], u1[:])
                h1 = tpool.tile([128, 512], dt, tag="h1")
                nc.vector.tensor_add(h1[:], w[:, 0:512], w[:, 512:1024])
                h2 = tpool.tile([128, 256], dt, tag="h2")
                nc.vector.tensor_add(h2[:], h1[:, 0:256], h1[:, 256:512])
                # reduce c0, write A with free = 16*c2 + 4*c1 + n0 (bf16;
                # only 4-16 values accumulate so bf16 rounding ~0.4% << tol)
                A = tpool.tile([128, 64], bf, tag="A")
                A2 = tpool.tile([128, 16], bf, tag="A2")
                A2r = A2[:].rearrange("p (c2 n2) -> p n2 c2", c2=4, n2=4)
                with nc.allow_low_precision(reason="bf16 matmul rhs, tol 2e-2"):
                    nc.vector.tensor_reduce(
                        A[:].rearrange("p (c2 c1 n0) -> p n0 c2 c1", c2=4, c1=4, n0=4),
                        h2[:].rearrange(
                            "p (n0 c2 c1 c0) -> p n0 c2 c1 c0", n0=4, c2=4, c1=4, c0=4
                        ),
                        axis=X, op=ADD,
                    )
                    for n2 in range(4):
                        rr = r01 if n2 < 2 else r23
                        nc.vector.tensor_reduce(
                            A2r[:, n2, :],
                            rr[:, 64 * (n2 % 2):64 * (n2 % 2) + 64].rearrange(
                                "p (nn c2) -> p c2 nn", nn=16, c2=4
                            ),
                            axis=X, op=ADD,
                        )

                # ---- L1: 16 routing matmuls -> c1p psum ----
                # strided out-AP so psum free = 256*n2 + 64*n0 + 16*c2 +
                # (4*c1 + c0): the value for output digits (n2,n0,c2,c1,c0)
                c1p = ppool.tile([128, 1024], dt, tag="c1p")
                c1pv = c1p[:].rearrange(
                    "p (n2 n0 c2 cc) -> p n2 n0 c2 cc", n2=4, n0=4, c2=4, cc=16
                )
                for n2o in range(4):
                    for c2o in range(4):
                        blk = n2o * 4 + c2o
                        nc.tensor.matmul(
                            c1pv[:, n2o, :, c2o, :],
                            s1sb[:, blk * 128:(blk + 1) * 128],
                            A[:, 0:64],
                            start=True, stop=True,
                        )
                # ---- L2: 4 reduce+broadcast matmuls -> gp psum ----
                # gp free = 16*c2o + (4*c1o + c0o); rhs j = (c2in, n2)
                gp = ppool.tile([128, 64], dt, tag="gp")
                for c2o in range(4):
                    nc.tensor.matmul(
                        gp[:, c2o * 16:(c2o + 1) * 16],
                        s2sb[:, c2o * 128:(c2o + 1) * 128],
                        A2[:, 0:16],
                        start=True, stop=True,
                    )
                c1ps.append(c1p)
                gps.append(gp)

            # ---- phase 2 (both groups): evac + flush, per n2-quarter.
            # Out-DMAs ride the same SWDGE ring as the inputs: FIFO
            # ordering guarantees they drain only after the full input
            # stream, with zero inter-chunk bubbles (each chunk's
            # descriptors are already queued when the previous chunk's
            # last descriptor retires).  g0 evacs all go to ACT (free
            # from ~23us); g1 alternates ACT/DVE so the last quarter's
            # evac — on the exec critical path — lands ~2x sooner. ----
            for g in range(NG):
                xt = xts_[g]
                c1p = c1ps[g]
                gp = gps[g]
                xtv = xt[:].rearrange(
                    "p (j i c) -> p j i c", j=64, i=3, c=64
                )
                c1e = c1p[:].rearrange(
                    "p (n2 o n0 c) -> p n2 o n0 c", n2=4, o=1, n0=4, c=64
                )
                xto = xt[:].rearrange(
                    "p (n2 n1 n0 i c) -> p n2 n1 n0 i c",
                    n2=4, n1=4, n0=4, i=3, c=64,
                )
                gpb = gp[:].rearrange("p (o c) -> p o c", o=1)
                outv = out[2 * g:2 * g + 2].rearrange(
                    "b (ph j) i c -> (b ph) (j i c)", ph=64
                )
                # chunks (0), (1), (2,3): the first quarter's evac is a
                # single 2us ACT op, so its descriptors are queued right
                # when the input stream drains (a half-chunk first would
                # leave a ~1.2us ring bubble); the rest amortizes into a
                # 2-quarter chunk to keep instruction count (and the
                # epilogue sem-clear chain, ~52ns/sem) low.
                for n2s in ((0,), (1,), (2, 3)):
                    for n2o in n2s:
                        use_dve = g > 0 and n2o % 2 == 1
                        cp = nc.vector.tensor_copy if use_dve else nc.scalar.copy
                        cp(
                            xto[:, n2o, :, :, 1, :],
                            c1e[:, n2o, :, :, :].broadcast_to((128, 4, 4, 64)),
                        )
                    jlo, jhi = 16 * n2s[0], 16 * n2s[-1] + 16
                    cp = (
                        nc.vector.tensor_copy
                        if (g > 0 and n2s[-1] % 2 == 1)
                        else nc.scalar.copy
                    )
                    cp(
                        xtv[:, jlo:jhi, 2, :],
                        gpb.broadcast_to((128, jhi - jlo, 64)),
                    )
                    nc.gpsimd.dma_start(
                        outv[:, 192 * jlo:192 * jhi],
                        xt[:, 192 * jlo:192 * jhi],
                    )
    # ---- startup surgery: hoist the 5 input-DMA triggers from the
    # user block into the init block, before the const-ap memsets and
    # the all-engine barrier.  They have no deps (DRAM x is valid from
    # t=0, the tile sems they bump are cleared by the compiler preamble
    # which still precedes everything on Pool), so gpsimd fires them
    # ~1.3us earlier — right when its compiler-injected preamble ends,
    # while the other engines are still at the barrier. ----
    blocks = nc.main_func.blocks
    b0, b1 = blocks[0], blocks[1]
    pool_dmas = [
        inst
        for inst in b1.instructions
        if isinstance(inst, mybir.InstDMACopy)
        and inst.engine == mybir.EngineType.Pool
    ][:3]
    for inst in pool_dmas:
        b1.instructions.remove(inst)
    memset_idx = next(
        i
        for i, inst in enumerate(b0.instructions)
        if isinstance(inst, mybir.InstMemset)
    )
    b0.instructions[memset_idx:memset_idx] = pool_dmas
    # Pool's init-block Drain would now wait ~36us for the hoisted
    # in-flight DMAs before the all-engine barrier, stalling every
    # engine's user code — drop it (it's a no-op when nothing is in
    # flight; the epilogue drains still guarantee completion).
    pool_drain = next(
        inst
        for inst in b0.instructions
        if isinstance(inst, mybir.InstDrain)
        and inst.engine == mybir.EngineType.Pool
    )
    b0.instructions.remove(pool_drain)

    # ---- epilogue surgery: between the two exit barriers Pool does a
    # redundant [drain, sem-range-clear] (~0.45us on the critical
    # path).  The compiler preamble re-clears the semaphore ranges at
    # the start of EVERY execution (the ~3us EVENT_SEMAPHOREs at t~0.3,
    # verified across repeated runs), so drop both; the final barrier
    # round (with its own drains) still guarantees completion. ----
    b2 = blocks[2]
    pool_drains_b2 = [
        inst
        for inst in b2.instructions
        if isinstance(inst, mybir.InstDrain)
        and inst.engine == mybir.EngineType.Pool
    ]
    assert len(pool_drains_b2) == 3
    b2.instructions.remove(pool_drains_b2[1])
    isa_clear = next(
        inst
        for inst in b2.instructions
        if type(inst).__name__ == "InstISA"
        and inst.engine == mybir.EngineType.Pool
    )
    b2.instructions.remove(isa_clear)
    # Strip the exit path down to Pool's ring drain ALONE.  The
    # barrier's DMA-completion detection rides semaphore propagation
    # (trailer desc -> sem write -> engine ES-wait, ~1us), which is
    # slower than the drain's own hardware ring-empty check; with the
    # barrier gone the drain starts polling right after Pool's last
    # trigger and ends when the ring actually empties.  All ES
    # broadcast/wait pairs are removed together (no dangling waits),
    # every other engine's exit drain is a no-op on an unused queue,
    # and NEFF completion still waits for every engine's stream end —
    # Pool's (the drain) is last, so outputs are guaranteed written.
    keep = pool_drains_b2[0]
    b2.instructions[:] = [keep]

    nc.compile()
    return nc


_NC_CACHE = {}


def _get_nc():
    if "nc" not in _NC_CACHE:
        _NC_CACHE["nc"] = _build_nc()
    return _NC_CACHE["nc"]


def _run(x, trace=False):
    from concourse.bass_utils import run_bass_kernel_spmd

    x = np.ascontiguousarray(x, dtype=np.float32)
    assert x.shape == (B_FULL, N, LVL, C), x.shape
    S12 = _make_s12()
    nc = _get_nc()
    in_maps = [
        {"x": np.ascontiguousarray(x[k * B_CORE:(k + 1) * B_CORE]),
         "s12": S12}
        for k in range(N_CORES)
    ]
    res = run_bass_kernel_spmd(nc, in_maps, list(range(N_CORES)), trace=trace)
    outs = [res.results[k]["out"] for k in range(N_CORES)]
    return np.ascontiguousarray(np.concatenate(outs, axis=0)), res


def kernel(**inputs: np.ndarray) -> np.ndarray:
    out, _ = _run(inputs["x"])
    return out


# revision 22
# speedup vs baseline: 1.1178x; 1.1178x over previous
"""Trainium2 Bass kernel for nn_Averager (pooling, 3-level box-average).

Math (verified vs reference): per sample, with input x[n, i, c] where
n = (n5 n4 n3 n2 n1 n0) base-4 digits, c = (c2 c1 c0) base-4 digits:
  out[:, :, 0, :] = x[:, :, 0, :]
  out1[n, c] = E[n4, n2, c2, c0, n0, c1],
      E[r5, r4, r3, r0; g2, g1] = mean over (n2, n1, c0) of x1
  out2[n, c] = G[c2, c1, c0],
      G[p, q, r] = mean over (n4, n3, n1, n0, c1, c0) of x2 with
      (n5, c2in, n2) = (p, q, r)

Sharding: data-parallel over batch, 4 samples per core on 8 cores,
processed as 2 groups of 2 samples.

Layout (pair-contiguous): SBUF partition p = b*64 + n//64 =
(b, n5, n4, n3); free j = n % 64 = 16*n2 + 4*n1 + n0, row (i, c).
A 6MB group is contiguous in DRAM and per-partition contiguous in
SBUF: each group is ONE 2-D in-DMA split in 2 j-halves.

DMA facts (measured): the 16 per-core queue engines cap at ~27GB/s
each regardless of descriptor size >=12KB (460/908/1814ns for
12/24/48KB), and the cost is set by the DRAM-side bytes (casting
DMAs don't reduce it).  Total in+out traffic 25.2MB/core = ~59us of
queue time — the pipeline floor.  256B-run patterns (per-level
regions) are descriptor-bound and hopeless.

ALL DMAs (in AND out) ride the single gpsimd SWDGE ring (Q0): the
ring is strict FIFO per queue engine, so output descriptors enqueue
behind the input stream and drain seamlessly the moment inputs
finish — no gate DMA, no HWDGE inter-chunk wait-for-complete stalls
(the old Sync HWDGE path serialized chunk N+1's enqueue on chunk N's
full completion, leaving 3-6us bubbles), and a straggling input
queue self-adjusts (out descriptors just queue up behind it).

More measured queue-engine facts (probe.py / microbench.py):
 - A DMA with descriptor count c spreads over min(16, 2^v2(c))
   queues (62 descs -> 2 queues at 31 each = serialized disaster!).
   Keep desc counts multiples of 16; uniform 1/16-per-queue shares
   are therefore unavoidable.  Each DMA also appends a 4B
   completion-sem descriptor on EVERY queue it used.
 - Setting qPoolDynamic num_queues=15 collapses per-queue bandwidth
   to ~17GB/s — do not touch.
 - E79 also hosts the dynamic rings' doorbell work and goes
   intermittently ~25% slow per BYTE (not per descriptor), +10-15us
   stragglers on a time-varying fraction of runs (external
   interference; always E79; hits all designs equally, unfixable in
   software since shares are uniform).
 - Startup is ~9.2us: ~7.4us Bass-init preamble (sem range clears,
   engine table loads, const memsets + all-engine barrier) before
   any user instruction, then ~0.7us/DMA descriptor-gen on gpsimd
   and ~1.4us doorbell->first-data latency.
 - The Tile epilogue sem-clear chain costs ~52ns per allocated
   semaphore serialized per engine — keep instruction count low
   (this is why selector expansion is 3 fused broadcast ops, not 20,
   and out-chunks are (0),(1),(2,3) not 4 quarters).
 - Healthy steady state after BIR surgery (see _build_nc tail):
   ~69.7us = 6.8us compiler preamble (gauge exec_time counts from the
   first branch at t~30; the ~3us per-engine sem-range clears are
   NEFF-level, untouchable from BIR) + 0.65 desc-gen + 0.45 doorbell
   + 60.1us gapless data window + ~1.3us ring-empty drain detection.
   gauge's exec endpoint lands just after the last real work, so the
   exit path is stripped to Pool's ring drain ALONE (no exit barriers
   at all): the drain's hardware ring-empty check (~0.93us detection
   latency after the last trailer) beats the barrier's semaphore-
   propagation path, and NEFF completion still waits for every
   engine's stream end.  Healthy exec: ~69.4-69.7us.

Engine split per group:
  DVE : L1 lane-local sums (u0/u1/w/h1/h2 -> A, bf16) and L2 sums
        (r01/r23 -> A2, bf16; split by j-half so the r01 reduce fills
        the stall between the two in-DMA halves), plus half of g1's
        PSUM evacuations (DVE is idle after stage-A; ACT alone made
        the last-chunk evac the exec tail).
  PE  : 16 L1 routing matmuls + 4 L2 reduce matmuls, 1-pass (lhsT =
        fp8e5 selectors — 1/64 and 1/4096 are exact powers of two —
        with bf16 rhs; fp32 would take 2 LDWEIGHTS+MATMUL passes).
        The L1 matmuls write PSUM through a strided out-AP so PSUM
        free = (n2, n0, c) matches the output interleave.
  ACT : PSUM evacuations (256B-contiguous runs, n1 via step-0 src dim)
        and the on-chip broadcast of the single L2 row G into all 64
        j-rows — runs parallel to DVE working on the next group.
Phase-1 (stage-A + matmuls, both groups) is emitted before phase-2
(evac + flush) so the framework's stage-reset barriers — which wait
on all earlier DMAs — never land inside the critical DVE chain.
Outputs are assembled IN-PLACE into the input tile (L0 rows pass
through untouched) and flushed in chunks (n2=0), (1), (2,3) as each
chunk's evac+broadcast completes: the first quarter's single 2us
evac is ready right as the input stream drains (12KB descriptors
are bandwidth-neutral), eliminating the transition bubble.
"""

import numpy as np

N_CORES = 8
B_FULL = 32
B_CORE = B_FULL // N_CORES  # 4
N = 4096
LVL = 3
C = 64


def _make_s12():
    """Compact selector factors, expanded on-chip (327KB -> 35KB DMA).

    S1[:, blk] = D1 * masks1[:, blk] and S2[:, c2o] = E2 * masks2[:, c2o]
    where the masks depend only on the partition index k.  All values are
    exact powers of two (or 0/1) so the fp8e5 multiplies are exact.

    Layout [128, 276] fp8e5: [0:128] D1 = (b==bm & k5==m4)/64,
    [128:256] E2 = (b==bm)/4096, [256:272] masks1 (blk = n2o*4+c2o),
    [272:276] masks2 (c2o).
    """
    import ml_dtypes

    k = np.arange(128)
    b, k5, k4, k3 = k >> 6, (k >> 4) & 3, (k >> 2) & 3, k & 3
    m = np.arange(128)
    bm, m4 = m >> 6, (m >> 2) & 3
    D1 = ((b[:, None] == bm[None, :]) & (k5[:, None] == m4[None, :])).astype(
        np.float32
    ) / 64.0
    E2 = (b[:, None] == bm[None, :]).astype(np.float32) / 4096.0
    masks1 = np.zeros((128, 16), np.float32)
    for n2o in range(4):
        for c2o in range(4):
            masks1[:, n2o * 4 + c2o] = ((k4 == n2o) & (k3 == c2o)).astype(
                np.float32
            )
    masks2 = np.zeros((128, 4), np.float32)
    for c2o in range(4):
        masks2[:, c2o] = (k5 == c2o).astype(np.float32)
    return np.ascontiguousarray(
        np.concatenate([D1, E2, masks1, masks2], axis=1).astype(
            ml_dtypes.float8_e5m2
        )
    )


def _build_nc():
    import concourse.bass as bass
    import concourse.tile as tile
    from concourse import mybir

    dt = mybir.dt.float32
    bf = mybir.dt.bfloat16
    f8 = mybir.dt.float8e5
    X = mybir.AxisListType.X
    ADD = mybir.AluOpType.add

    from concourse import bacc
    nc = bacc.Bacc()
    x = nc.declare_dram_parameter("x", [B_CORE, N, LVL, C], dt, isOutput=False)
    s12 = nc.declare_dram_parameter("s12", [128, 276], f8, isOutput=False)
    out = nc.declare_dram_parameter("out", [B_CORE, N, LVL, C], dt, isOutput=True)

    NG = B_CORE // 2

    with tile.TileContext(nc) as tc:
        with (
            tc.tile_pool(name="consts", bufs=1) as cpool,
            tc.tile_pool(name="xin", bufs=2) as xpool,
            tc.tile_pool(name="tmp", bufs=1) as tpool,
            tc.tile_pool(name="psum", bufs=2, space="PSUM") as ppool,
        ):
            # ---- all input DMAs upfront: x group 0 halves, selectors,
            # x group 1 halves, drained back-to-back in this order
            # (selector-first was tried and measured ~1.5us WORSE on
            # healthy runs — keep s12c third) ----
            # In-DMAs as j-halves (24KB descs), NOT full tiles: 48KB
            # descriptors are 0.4% faster per byte but delay stage-A
            # from h0-complete (~16us) to tile-complete (~24us), which
            # measured as a 6.7us ring bubble at the in->out
            # transition.  The half split staggers the DVE chain early.
            xts_ = []
            for g in range(NG):
                xt = xpool.tile([128, 12288], dt, tag="xt")
                xsrc = x[2 * g:2 * g + 2].rearrange(
                    "b (ph j) i c -> (b ph) (j i c)", ph=64
                )
                nc.gpsimd.dma_start(xt[:, 0:6144], xsrc[:, 0:6144])
                nc.gpsimd.dma_start(xt[:, 6144:12288], xsrc[:, 6144:12288])
                xts_.append(xt)
                if g == 0:
                    s12c = cpool.tile([128, 276], f8, tag="s12c")
                    nc.gpsimd.dma_start(s12c[:], s12[:])

            # ---- expand compact selector factors on DVE (idle until the
            # first x half lands): S1 blk = D1 * mask1[blk], S2 c2o =
            # E2 * mask2[c2o].  Exact powers of two -> fp8e5 exact. ----
            ssb = cpool.tile([128, 2560], f8, tag="ssb")
            s1sb = ssb[:, 0:2048]
            s2sb = ssb[:, 2048:2560]
            maskf = cpool.tile([128, 20], dt, tag="maskf")
            nc.vector.tensor_copy(maskf[:], s12c[:, 256:276])
            nc.vector.tensor_mul(
                s1sb.rearrange("p (blk m) -> p blk m", blk=16, m=128),
                s12c[:, 0:128].rearrange(
                    "p (o m) -> p o m", o=1).broadcast_to((128, 16, 128)),
                maskf[:, 0:16].rearrange(
                    "p (blk o) -> p blk o", o=1).broadcast_to((128, 16, 128)),
            )
            nc.vector.tensor_mul(
                s2sb.rearrange("p (blk m) -> p blk m", blk=4, m=128),
                s12c[:, 128:256].rearrange(
                    "p (o m) -> p o m", o=1).broadcast_to((128, 4, 128)),
                maskf[:, 16:20].rearrange(
                    "p (blk o) -> p blk o", o=1).broadcast_to((128, 4, 128)),
            )

            # ---- phase 1 (both groups): DVE stage-A + PE matmuls.
            # Emitted before any out-DMA so the framework's stage-reset
            # barriers (which wait on all earlier DMAs) never insert an
            # out-DMA wait into the critical DVE chain. ----
            c1ps, gps = [], []
            for g in range(NG):
                xt = xts_[g]
                v = xt[:].rearrange(
                    "p (n2 n1 n0 i c) -> p n2 n1 n0 i c",
                    n2=4, n1=4, n0=4, i=3, c=64,
                )
                xw = xt[:].rearrange(
                    "p (j i c2 cc) -> p j i c2 cc", j=64, i=3, c2=4, cc=16
                )

                # ---- stage A per j-half (each needs only that in-half):
                # L1: u = fold n2-pairs of i=1 rows
                # L2: r = fold (c1 c0) of i=2 rows ----
                u0 = tpool.tile([128, 1024], dt, tag="u0")
                nc.vector.tensor_add(
                    u0[:].rearrange("p (n1 n0 c) -> p n1 n0 c", n1=4, n0=4, c=64),
                    v[:, 0, :, :, 1, :], v[:, 1, :, :, 1, :],
                )
                r01 = tpool.tile([128, 128], dt, tag="r01")
                nc.vector.tensor_reduce(
                    r01[:].rearrange("p (j c2) -> p j c2", j=32, c2=4),
                    xw[:, 0:32, 2, :, :],
                    axis=X, op=ADD,
                )
                u1 = tpool.tile([128, 1024], dt, tag="u1")
                nc.vector.tensor_add(
                    u1[:].rearrange("p (n1 n0 c) -> p n1 n0 c", n1=4, n0=4, c=64),
                    v[:, 2, :, :, 1, :], v[:, 3, :, :, 1, :],
                )
                r23 = tpool.tile([128, 128], dt, tag="r23")
                nc.vector.tensor_reduce(
                    r23[:].rearrange("p (j c2) -> p j c2", j=32, c2=4),
                    xw[:, 32:64, 2, :, :],
                    axis=X, op=ADD,
                )

                # ---- cross-half folds -> A (L1) and A2 (L2), bf16 ----
                w = tpool.tile([128, 1024], dt, tag="w")
                nc.vector.tensor_add(w[:], u0[:], u1[:])
                h1 = tpool.tile([128, 512], dt, tag="h1")
                nc.vector.tensor_add(h1[:], w[:, 0:512], w[:, 512:1024])
                h2 = tpool.tile([128, 256], dt, tag="h2")
                nc.vector.tensor_add(h2[:], h1[:, 0:256], h1[:, 256:512])
                # reduce c0, write A with free = 16*c2 + 4*c1 + n0 (bf16;
                # only 4-16 values accumulate so bf16 rounding ~0.4% << tol)
                A = tpool.tile([128, 64], bf, tag="A")
                A2 = tpool.tile([128, 16], bf, tag="A2")
                A2r = A2[:].rearrange("p (c2 n2) -> p n2 c2", c2=4, n2=4)
                with nc.allow_low_precision(reason="bf16 matmul rhs, tol 2e-2"):
                    nc.vector.tensor_reduce(
                        A[:].rearrange("p (c2 c1 n0) -> p n0 c2 c1", c2=4, c1=4, n0=4),
                        h2[:].rearrange(
                            "p (n0 c2 c1 c0) -> p n0 c2 c1 c0", n0=4, c2=4, c1=4, c0=4
                        ),
                        axis=X, op=ADD,
                    )
                    for n2 in range(4):
                        rr = r01 if n2 < 2 else r23
                        nc.vector.tensor_reduce(
                            A2r[:, n2, :],
                            rr[:, 64 * (n2 % 2):64 * (n2 % 2) + 64].rearrange(
                                "p (nn c2) -> p c2 nn", nn=16, c2=4
                            ),
                            axis=X, op=ADD,
                        )

                # ---- L1: 16 routing matmuls -> c1p psum ----
                # strided out-AP so psum free = 256*n2 + 64*n0 + 16*c2 +
                # (4*c1 + c0): the value for output digits (n2,n0,c2,c1,c0)
                c1p = ppool.tile([128, 1024], dt, tag="c1p")
                c1pv = c1p[:].rearrange(
                    "p (n2 n0 c2 cc) -> p n2 n0 c2 cc", n2=4, n0=4, c2=4, cc=16
                )
                for n2o in range(4):
                    for c2o in range(4):
                        blk = n2o * 4 + c2o
                        nc.tensor.matmul(
                            c1pv[:, n2o, :, c2o, :],
                            s1sb[:, blk * 128:(blk + 1) * 128],
                            A[:, 0:64],
                            start=True, stop=True,
                        )
                # ---- L2: 4 reduce+broadcast matmuls -> gp psum ----
                # gp free = 16*c2o + (4*c1o + c0o); rhs j = (c2in, n2)
                gp = ppool.tile([128, 64], dt, tag="gp")
                for c2o in range(4):
                    nc.tensor.matmul(
                        gp[:, c2o * 16:(c2o + 1) * 16],
                        s2sb[:, c2o * 128:(c2o + 1) * 128],
                        A2[:, 0:16],
                        start=True, stop=True,
                    )
                c1ps.append(c1p)
                gps.append(gp)

            # ---- phase 2 (both groups): evac + flush, per n2-quarter.
            # Out-DMAs ride the same SWDGE ring as the inputs: FIFO
            # ordering guarantees they drain only after the full input
            # stream, with zero inter-chunk bubbles (each chunk's
            # descriptors are already queued when the previous chunk's
            # last descriptor retires).  g0 evacs all go to ACT (free
            # from ~23us); g1 alternates ACT/DVE so the last quarter's
            # evac — on the exec critical path — lands ~2x sooner. ----
            for g in range(NG):
                xt = xts_[g]
                c1p = c1ps[g]
                gp = gps[g]
                xtv = xt[:].rearrange(
                    "p (j i c) -> p j i c", j=64, i=3, c=64
                )
                c1e = c1p[:].rearrange(
                    "p (n2 o n0 c) -> p n2 o n0 c", n2=4, o=1, n0=4, c=64
                )
                xto = xt[:].rearrange(
                    "p (n2 n1 n0 i c) -> p n2 n1 n0 i c",
                    n2=4, n1=4, n0=4, i=3, c=64,
                )
                gpb = gp[:].rearrange("p (o c) -> p o c", o=1)
                outv = out[2 * g:2 * g + 2].rearrange(
                    "b (ph j) i c -> (b ph) (j i c)", ph=64
                )
                # chunks (0), (1), (2,3): the first quarter's evac is a
                # single 2us ACT op, so its descriptors are queued right
                # when the input stream drains (a half-chunk first would
                # leave a ~1.2us ring bubble); the rest amortizes into a
                # 2-quarter chunk to keep instruction count (and the
                # epilogue sem-clear chain, ~52ns/sem) low.
                for n2s in ((0,), (1,), (2, 3)):
                    for n2o in n2s:
                        use_dve = g > 0 and n2o % 2 == 1
                        cp = nc.vector.tensor_copy if use_dve else nc.scalar.copy
                        cp(
                            xto[:, n2o, :, :, 1, :],
                            c1e[:, n2o, :, :, :].broadcast_to((128, 4, 4, 64)),
                        )
                    jlo, jhi = 16 * n2s[0], 16 * n2s[-1] + 16
                    cp = (
                        nc.vector.tensor_copy
                        if (g > 0 and n2s[-1] % 2 == 1)
                        else nc.scalar.copy
                    )
                    cp(
                        xtv[:, jlo:jhi, 2, :],
                        gpb.broadcast_to((128, jhi - jlo, 64)),
                    )
                    nc.gpsimd.dma_start(
                        outv[:, 192 * jlo:192 * jhi],
                        xt[:, 192 * jlo:192 * jhi],
                    )
    # ---- startup surgery: hoist the 5 input-DMA triggers from the
    # user block into the init block, before the const-ap memsets and
    # the all-engine barrier.  They have no deps (DRAM x is valid from
    # t=0, the tile sems they bump are cleared by the compiler preamble
    # which still precedes everything on Pool), so gpsimd fires them
    # ~1.3us earlier — right when its compiler-injected preamble ends,
    # while the other engines are still at the barrier. ----
    blocks = nc.main_func.blocks
    b0, b1 = blocks[0], blocks[1]
    pool_dmas = [
        inst
        for inst in b1.instructions
        if isinstance(inst, mybir.InstDMACopy)
        and inst.engine == mybir.EngineType.Pool
    ][:5]
    for inst in pool_dmas:
        b1.instructions.remove(inst)
    memset_idx = next(
        i
        for i, inst in enumerate(b0.instructions)
        if isinstance(inst, mybir.InstMemset)
    )
    b0.instructions[memset_idx:memset_idx] = pool_dmas
    # Pool's init-block Drain would now wait ~36us for the hoisted
    # in-flight DMAs before the all-engine barrier, stalling every
    # engine's user code — drop it (it's a no-op when nothing is in
    # flight; the epilogue drains still guarantee completion).
    pool_drain = next(
        inst
        for inst in b0.instructions
        if isinstance(inst, mybir.InstDrain)
        and inst.engine == mybir.EngineType.Pool
    )
    b0.instructions.remove(pool_drain)

    # ---- epilogue surgery: between the two exit barriers Pool does a
    # redundant [drain, sem-range-clear] (~0.45us on the critical
    # path).  The compiler preamble re-clears the semaphore ranges at
    # the start of EVERY execution (the ~3us EVENT_SEMAPHOREs at t~0.3,
    # verified across repeated runs), so drop both; the final barrier
    # round (with its own drains) still guarantees completion. ----
    b2 = blocks[2]
    pool_drains_b2 = [
        inst
        for inst in b2.instructions
        if isinstance(inst, mybir.InstDrain)
        and inst.engine == mybir.EngineType.Pool
    ]
    assert len(pool_drains_b2) == 3
    b2.instructions.remove(pool_drains_b2[1])
    isa_clear = next(
        inst
        for inst in b2.instructions
        if type(inst).__name__ == "InstISA"
        and inst.engine == mybir.EngineType.Pool
    )
    b2.instructions.remove(isa_clear)
    # Strip the exit path down to Pool's ring drain ALONE.  The
    # barrier's DMA-completion detection rides semaphore propagation
    # (trailer desc -> sem write -> engine ES-wait, ~1us), which is
    # slower than the drain's own hardware ring-empty check; with the
    # barrier gone the drain starts polling right after Pool's last
    # trigger and ends when the ring actually empties.  All ES
    # broadcast/wait pairs are removed together (no dangling waits),
    # every other engine's exit drain is a no-op on an unused queue,
    # and NEFF completion still waits for every engine's stream end —
    # Pool's (the drain) is last, so outputs are guaranteed written.
    keep = pool_drains_b2[0]
    b2.instructions[:] = [keep]

    nc.compile()
    return nc


_NC_CACHE = {}


def _get_nc():
    if "nc" not in _NC_CACHE:
        _NC_CACHE["nc"] = _build_nc()
    return _NC_CACHE["nc"]


def _run(x, trace=False):
    from concourse.bass_utils import run_bass_kernel_spmd

    x = np.ascontiguousarray(x, dtype=np.float32)
    assert x.shape == (B_FULL, N, LVL, C), x.shape
    S12 = _make_s12()
    nc = _get_nc()
    in_maps = [
        {"x": np.ascontiguousarray(x[k * B_CORE:(k + 1) * B_CORE]),
         "s12": S12}
        for k in range(N_CORES)
    ]
    res = run_bass_kernel_spmd(nc, in_maps, list(range(N_CORES)), trace=trace)
    outs = [res.results[k]["out"] for k in range(N_CORES)]
    return np.ascontiguousarray(np.concatenate(outs, axis=0)), res


def kernel(**inputs: np.ndarray) -> np.ndarray:
    out, _ = _run(inputs["x"])
    return out
